# revision 43
# baseline (speedup 1.0000x reference)
# Trainium2 Bass/Tile kernel for causal GQA attention (dense_transformer).
#
# Reference computation (fp32):
#   Q = x@wq, K = x@wk, V = x@wv  (rotary on Q,K; GQA 32 q heads / 8 kv heads)
#   out = softmax(QK^T/sqrt(64), causal) @ V @ wo
#
# Sharding: tensor-parallel over heads (TP=4: 8 q heads + 2 kv heads per
# core) x data-parallel over batch (DP=2: 2 batches per core) = 8 cores.
# Each core computes a partial [2,1024,2048] output (its heads' wo
# contribution); host sums partials within each DP group.
#
# Device pipeline per core (all matmuls bf16 -> fp32 PSUM), 3-deep
# software pipeline (batch-interleaved tile order). Within a step the
# previous tile's score pairs are interleaved between projection halves
# so the ACT exp burst (the per-step serializer) always has PE work to
# hide behind:
#   [2a(i-1,p0) | projQ/KV(i) 1st half | 2a(p1) | 2nd half | 2a(p2) |
#    rotary+transposes(i) | 2a(p3) | 2b(i-2)]
#   phase1: Q and fused K|V projections (Q and KV matmul streams
#     separated so LDWEIGHTS overlaps the previous moving stream), from
#     host-pretransposed x^T tiles; xt DMA prefetched one step ahead on
#     the gpsimd queue; all weights preloaded upfront across 3 DMA
#     queues (wq first - its stream runs first). Rotary on Q/K via DVE
#     in natural layout, PE-transpose Q/K to head-major [d, q], V kept
#     natural [k, dv] with an appended ones column (softmax denom).
#   phase2a: scoresT = K^T-chunk.T @ Q^T, two heads of a pair as
#     row-tiled (PE-array-packed) matmuls into separate PSUM banks
#     (psc=3 so the PE runs ahead of the ACT exp drain); exp on ACT (no
#     max subtraction - scores are bounded ~6); diagonal causal mask on
#     GpSimd mid-pipeline (latency-tolerant), on DVE in the epilogue
#     (latency-critical).
#   phase2b: AV accumulation (P^T stationary; ones column yields the
#     softmax denominator), one reciprocal + per-partition scale,
#     PE-transpose to [d, q], W_O accumulation over pairs; y written
#     bf16 (host gather upcasts and sums TP partials).
# Epilogue interleaves the last tile's score pairs with the previous
# tile's AV chains to shorten the drain.
import numpy as np
import ml_dtypes

B, S, D = 4, 1024, 2048
NH, NKV, HD = 32, 8, 64
TP, DP = 4, 2
QH = NH // TP            # 8 q heads per core
KVH = NKV // TP          # 2 kv heads per core
BL = B // DP             # 2 batches per core
NT = S // 128            # 8 s-tiles per batch
NQT = BL * NT            # 16 q-tiles per core
DC = D // 128            # 16 contraction chunks for the projections
DC8 = D // 256           # 8 fp8 DoubleRow chunks (256 contraction rows each)
W8SCALE = 64.0           # fp8 weight pre-scale (undone via cos/sin tables)
PAIRS = QH // 2          # 4 head pairs (h, h+4) packed per 128 partitions
SCALE = 1.0 / float(np.sqrt(HD))
PERM = [0, 4, 1, 5, 2, 6, 3, 7]   # local head order: pair p = (p, p+4)

bf = ml_dtypes.bfloat16
f8 = ml_dtypes.float8_e4m3

_built = None


def _build():
    from contextlib import ExitStack
    import concourse.bacc as bacc
    import concourse.tile as tile
    from concourse import mybir

    f32 = mybir.dt.float32
    b16 = mybir.dt.bfloat16
    fp8 = mybir.dt.float8e4
    DR = mybir.MatmulPerfMode.DoubleRow
    Exp = mybir.ActivationFunctionType.Exp

    nc = bacc.Bacc("TRN2", target_bir_lowering=False, debug=False,
                   num_devices=TP * DP)

    xt_d = nc.dram_tensor("xt", [NQT, 128, DC, 128], b16, kind="ExternalInput").ap()
    wq_d = nc.dram_tensor("wqr", [DC, 128, QH * HD], b16, kind="ExternalInput").ap()
    wkv_d = nc.dram_tensor("wkvr", [DC, 128, 2 * KVH * HD], b16, kind="ExternalInput").ap()
    wo_d = nc.dram_tensor("wor", [128, PAIRS, D], b16, kind="ExternalInput").ap()
    cos_d = nc.dram_tensor("cosr", [NT, 128, QH * HD // 2], f32, kind="ExternalInput").ap()
    sin_d = nc.dram_tensor("sinr", [NT, 128, QH * HD // 2], f32, kind="ExternalInput").ap()
    mask_d = nc.dram_tensor("maskr", [128, 128], b16, kind="ExternalInput").ap()
    id_d = nc.dram_tensor("identr", [128, 128], b16, kind="ExternalInput").ap()
    y_d = nc.dram_tensor("y", [NQT, 128, D], b16, kind="ExternalOutput").ap()

    with tile.TileContext(nc) as tc:
        with ExitStack() as ctx:
            singles = ctx.enter_context(tc.tile_pool(name="singles", bufs=1))
            # PSUM: 8 banks total: 2 proj + 3 scores/tq + 3 av/tr/wo
            pp = ctx.enter_context(tc.tile_pool(name="pp", bufs=3, space="PSUM"))
            psc = ctx.enter_context(tc.tile_pool(name="psc", bufs=3, space="PSUM"))
            pav = ctx.enter_context(tc.tile_pool(name="pav", bufs=2, space="PSUM"))
            xtp = ctx.enter_context(tc.tile_pool(name="xtp", bufs=4))
            rot = ctx.enter_context(tc.tile_pool(name="rot", bufs=2))
            rtmp = ctx.enter_context(tc.tile_pool(name="rtmp", bufs=4))
            persist = ctx.enter_context(tc.tile_pool(name="persist", bufs=1))
            ptp = ctx.enter_context(tc.tile_pool(name="ptp", bufs=1))
            anp = ctx.enter_context(tc.tile_pool(name="anp", bufs=6))
            atp = ctx.enter_context(tc.tile_pool(name="atp", bufs=10))
            outp = ctx.enter_context(tc.tile_pool(name="outp", bufs=3))

            mask_sb = singles.tile([128, 128], b16)
            nc.sync.dma_start(out=mask_sb, in_=mask_d)
            ident_sb = singles.tile([128, 128], b16)
            nc.sync.dma_start(out=ident_sb, in_=id_d)
            # weight/cos tiles created here; DMAs emitted lazily inside
            # phase1 so they interleave with xt loads on the sync queue
            wq_c, wkv_c, cos_c, sin_c = [], [], [], []
            for c in range(DC):
                wq_c.append(singles.tile([128, QH * HD], b16, name=f"wq{c}"))
                wkv_c.append(singles.tile([128, 2 * KVH * HD], b16, name=f"wkv{c}"))
                if c < NT:
                    cos_c.append(singles.tile([128, QH * HD // 2], f32, name=f"cos{c}"))
                    sin_c.append(singles.tile([128, QH * HD // 2], f32, name=f"sin{c}"))
            wo_sb = singles.tile([128, PAIRS, D], b16)
            loaded = set()

            xt_tiles = {}
            qt_tiles = {}
            kt_tiles = {}
            v_tiles = {}
            pt_tiles = {}

            def prefetch_xt(i):
                xt_sb = xtp.tile([128, DC, 128], b16, tag="xt", name=f"xt{i}")
                nc.gpsimd.dma_start(out=xt_sb, in_=xt_d[i])
                xt_tiles[i] = xt_sb

            proj_state = {}

            def proj_part(i, h):
                if h == 0:
                    xt_sb = xt_tiles.pop(i)
                    q_ps = pp.tile([128, QH * HD], f32, tag="pp", name=f"qps{i}")
                    kv_ps = pp.tile([128, 2 * KVH * HD], f32, tag="pp",
                                    name=f"kvps{i}")
                    proj_state[i] = (xt_sb, q_ps, kv_ps)
                xt_sb, q_ps, kv_ps = proj_state[i]
                lo, hi = (0, DC // 2) if h == 0 else (DC // 2, DC)
                # separate Q and KV streams so LDW stays overlapped with the
                # previous matmul's moving stream
                for c in range(lo, hi):
                    nc.tensor.matmul(q_ps, xt_sb[:, c, :], wq_c[c],
                                     start=(c == 0), stop=(c == DC - 1))
                for c in range(lo, hi):
                    nc.tensor.matmul(kv_ps, xt_sb[:, c, :], wkv_c[c],
                                     start=(c == 0), stop=(c == DC - 1))

            def rot_tr(i):
                bl, t = divmod(i, NT)
                xt_sb, q_ps, kv_ps = proj_state.pop(i)

                # ---------- rotary (natural layout, pairs on free dim) ----
                c_sl = cos_c[t]
                s_sl = sin_c[t]
                qrot = rot.tile([128, QH * HD], b16, tag="qrot", name=f"qr{i}")
                qv = qrot.rearrange("p (n two) -> p two n", two=2)
                qp = q_ps.rearrange("p (n two) -> p two n", two=2)
                t1 = rtmp.tile([128, QH * HD // 2], f32, tag="t1", name=f"t1a{i}")
                t2 = rtmp.tile([128, QH * HD // 2], f32, tag="t2", name=f"t2a{i}")
                nc.vector.tensor_mul(t1, qp[:, 0, :], c_sl)
                nc.vector.tensor_mul(t2, qp[:, 1, :], s_sl)
                nc.vector.tensor_sub(qv[:, 0, :], t1, t2)
                t3 = rtmp.tile([128, QH * HD // 2], f32, tag="t1", name=f"t1b{i}")
                t4 = rtmp.tile([128, QH * HD // 2], f32, tag="t2", name=f"t2b{i}")
                nc.vector.tensor_mul(t3, qp[:, 0, :], s_sl)
                nc.vector.tensor_mul(t4, qp[:, 1, :], c_sl)
                nc.vector.tensor_add(qv[:, 1, :], t3, t4)

                ck_sl = cos_c[t][:, 0:KVH * HD // 2]
                sk_sl = sin_c[t][:, 0:KVH * HD // 2]
                krot = rot.tile([128, KVH * HD], b16, tag="krot", name=f"kr{i}")
                kv_ = krot.rearrange("p (n two) -> p two n", two=2)
                kp = kv_ps[:, 0:KVH * HD].rearrange("p (n two) -> p two n", two=2)
                u1 = rtmp.tile([128, KVH * HD // 2], f32, tag="u1", name=f"u1a{i}")
                u2 = rtmp.tile([128, KVH * HD // 2], f32, tag="u2", name=f"u2a{i}")
                nc.vector.tensor_mul(u1, kp[:, 0, :], ck_sl)
                nc.vector.tensor_mul(u2, kp[:, 1, :], sk_sl)
                nc.vector.tensor_sub(kv_[:, 0, :], u1, u2)
                u3 = rtmp.tile([128, KVH * HD // 2], f32, tag="u1", name=f"u1b{i}")
                u4 = rtmp.tile([128, KVH * HD // 2], f32, tag="u2", name=f"u2b{i}")
                nc.vector.tensor_mul(u3, kp[:, 0, :], sk_sl)
                nc.vector.tensor_mul(u4, kp[:, 1, :], ck_sl)
                nc.vector.tensor_add(kv_[:, 1, :], u3, u4)

                # ---------- V with ones column per kv head ----------
                v_sb = persist.tile([128, KVH * (HD + 1)], b16, tag="v",
                                    bufs=2 * NT + 4, name=f"v{i}")
                voff = KVH * HD
                nc.vector.tensor_copy(v_sb[:, 0:HD], kv_ps[:, voff:voff + HD])
                nc.vector.tensor_copy(v_sb[:, HD + 1:2 * HD + 1],
                                      kv_ps[:, voff + HD:voff + 2 * HD])
                ones_v = v_sb.rearrange("p (h x) -> p h x", x=HD + 1)[:, :, HD:HD + 1]
                nc.vector.memset(ones_v, 1.0)
                v_tiles[i] = v_sb

                # ---------- PE transposes to [d, q] ----------
                # all pairs into one contiguous tile: head-A rows of every
                # pair form a [64, 4, 128] view for wide score streams
                qt_all = persist.tile([128, PAIRS, 128], b16, tag="qt",
                                      bufs=20, name=f"qtt{i}")
                for j in range(PAIRS):
                    tp_ps = psc.tile([128, 128], b16, tag="sc", name=f"tq{i}_{j}")
                    nc.tensor.transpose(tp_ps, qrot[:, j * 128:(j + 1) * 128],
                                        ident_sb)
                    nc.vector.tensor_copy(qt_all[:, j, :], tp_ps)
                qt_tiles[i] = qt_all
                kp_ps = psc.tile([128, 128], b16, tag="sc", name=f"tk{i}")
                nc.tensor.transpose(kp_ps, krot, ident_sb)
                kt_t = persist.tile([128, 128], b16, tag="kt", bufs=2 * NT + 4,
                                    name=f"ktt{i}")
                nc.vector.tensor_copy(kt_t, kp_ps)
                kt_tiles[i] = kt_t

            def phase2a(i, part=None, meng=None):
                # part in 0..3 selects a quarter of this tile's k-chunks so
                # score work interleaves between projection halves
                meng = meng or nc.gpsimd
                bl, t = divmod(i, NT)
                ib = bl * NT
                nkc = t + 1
                qtt = qt_tiles[i]
                parts = range(4) if part is None else [part]
                for s in parts:
                    c0 = (nkc * s + 3) // 4
                    c1 = (nkc * (s + 1) + 3) // 4
                    for kc in range(c0, c1):
                        ktt = kt_tiles[ib + kc]
                        scA = psc.tile([128, PAIRS * 128], f32, tag="sc",
                                       name=f"scA{i}_{kc}")
                        scB = psc.tile([128, PAIRS * 128], f32, tag="sc",
                                       name=f"scB{i}_{kc}")
                        nc.tensor.matmul(scA, ktt[0:64, :], qtt[0:64, :, :],
                                         start=True, stop=True)
                        nc.tensor.matmul(scB, ktt[64:128, :], qtt[64:128, :, :],
                                         start=True, stop=True)
                        ptA = ptp.tile([128, PAIRS * 128], b16, tag="pt", bufs=40,
                                       name=f"ptA{i}_{kc}")
                        ptB = ptp.tile([128, PAIRS * 128], b16, tag="pt", bufs=40,
                                       name=f"ptB{i}_{kc}")
                        nc.scalar.activation(ptA, scA, Exp, scale=SCALE)
                        nc.scalar.activation(ptB, scB, Exp, scale=SCALE)
                        if kc == t:   # diagonal chunk: causal mask per q-block
                            for j in range(PAIRS):
                                o = j * 128
                                meng.tensor_mul(ptA[:, o:o + 128],
                                                ptA[:, o:o + 128], mask_sb)
                                meng.tensor_mul(ptB[:, o:o + 128],
                                                ptB[:, o:o + 128], mask_sb)
                        pt_tiles[(i, kc)] = (ptA, ptB)

            def phase2b_av(i, p):
                bl, t = divmod(i, NT)
                ib = bl * NT
                nkc = t + 1
                if True:
                    avA = pav.tile([128, HD + 1], f32, tag="av", name=f"avA{i}_{p}")
                    avB = pav.tile([128, HD + 1], f32, tag="av", name=f"avB{i}_{p}")
                    o = p * 128
                    for kc in range(nkc):
                        ptA, ptB = (pt_tiles.pop((i, kc)) if p == PAIRS - 1
                                    else pt_tiles[(i, kc)])
                        vt = v_tiles[ib + kc]
                        nc.tensor.matmul(avA, ptA[:, o:o + 128],
                                         vt[:, 0:HD + 1],
                                         start=(kc == 0), stop=(kc == t))
                        nc.tensor.matmul(avB, ptB[:, o:o + 128],
                                         vt[:, HD + 1:2 * (HD + 1)],
                                         start=(kc == 0), stop=(kc == t))
                    rA = anp.tile([128, 1], f32, tag="recip", name=f"rA{i}_{p}")
                    rB = anp.tile([128, 1], f32, tag="recip", name=f"rB{i}_{p}")
                    nc.vector.reciprocal(rA, avA[:, HD:HD + 1])
                    nc.vector.reciprocal(rB, avB[:, HD:HD + 1])
                    atAB = anp.tile([128, 2 * HD], b16, tag="attn", name=f"aAB{i}_{p}")
                    nc.vector.tensor_scalar_mul(atAB[:, 0:HD], avA[:, 0:HD], rA)
                    nc.vector.tensor_scalar_mul(atAB[:, HD:2 * HD], avB[:, 0:HD], rB)
                    return atAB

            def phase2b_out(i, attn_sb):
                attnT = []
                for p in range(PAIRS):
                    tr_ps = pav.tile([128, 128], b16, tag="av", name=f"trp{i}_{p}")
                    nc.tensor.transpose(tr_ps, attn_sb[p], ident_sb)
                    aT = atp.tile([128, 128], b16, tag="att", name=f"aT{i}_{p}")
                    # split the copy chain across two engines so the wo
                    # matmuls see all four attnT tiles sooner
                    if p % 2 == 0:
                        nc.vector.tensor_copy(aT, tr_ps)
                    else:
                        nc.scalar.copy(aT, tr_ps)
                    attnT.append(aT)
                out_sb = outp.tile([128, D], b16, tag="out", name=f"o{i}")
                for n in range(4):
                    wo_ps = pav.tile([128, 512], f32, tag="av", name=f"wops{i}_{n}")
                    for p in range(PAIRS):
                        nc.tensor.matmul(wo_ps, attnT[p],
                                         wo_sb[:, p, n * 512:(n + 1) * 512],
                                         start=(p == 0), stop=(p == PAIRS - 1))
                    sl = slice(n * 512, (n + 1) * 512)
                    if n < 2:
                        nc.scalar.copy(out_sb[:, sl], wo_ps)
                    else:
                        nc.vector.tensor_copy(out_sb[:, sl], wo_ps)
                    nc.sync.dma_start(out=y_d[i][:, sl], in_=out_sb[:, sl])

            def phase2b(i):
                phase2b_out(i, [phase2b_av(i, p) for p in range(PAIRS)])

            # 3-deep software pipeline, batches interleaved so two
            # independent tile streams are always in flight
            order = []
            for t in range(NT):
                order.append(t)
                order.append(NT + t)
            prefetch_xt(order[0])
            qs = [nc.scalar, nc.sync, nc.gpsimd]
            for c in range(DC):
                qs[c % 3].dma_start(out=wq_c[c], in_=wq_d[c])
            qs[1].dma_start(out=cos_c[0], in_=cos_d[0])
            qs[2].dma_start(out=sin_c[0], in_=sin_d[0])
            for c in range(DC):
                qs[(c + 1) % 3].dma_start(out=wkv_c[c], in_=wkv_d[c])
            for c in range(1, NT):
                qs[c % 3].dma_start(out=cos_c[c], in_=cos_d[c])
                qs[(c + 1) % 3].dma_start(out=sin_c[c], in_=sin_d[c])
            nc.scalar.dma_start(out=wo_sb, in_=wo_d)
            # p-state warmup: dummy matmuls fill the weight-DMA wait so the
            # PE reaches full clock before the first real projection
            warm = psc.tile([128, 128], f32, tag="sc", name="warm")
            for _ in range(72):
                nc.tensor.matmul(warm, ident_sb, ident_sb, start=True, stop=True)
            for idx, i in enumerate(order):
                if idx + 1 < len(order):
                    prefetch_xt(order[idx + 1])
                prev = order[idx - 1] if idx >= 1 else None
                if prev is not None:
                    phase2a(prev, 0)
                proj_part(i, 0)
                if prev is not None:
                    phase2a(prev, 1)
                proj_part(i, 1)
                if prev is not None:
                    phase2a(prev, 2)
                rot_tr(i)
                if prev is not None:
                    phase2a(prev, 3)
                if idx == len(order) - 1:
                    phase2a(i, 0, meng=nc.vector)
                if idx >= 2:
                    phase2b(order[idx - 2])
            last = order[-1]
            prev2 = order[-2]
            a14 = []
            for p in range(PAIRS):
                if p + 1 < PAIRS:
                    phase2a(last, p + 1, meng=nc.vector)
                a14.append(phase2b_av(prev2, p))
            phase2b_out(prev2, a14)
            a15 = [phase2b_av(last, p) for p in range(PAIRS)]
            phase2b_out(last, a15)

    nc.compile()
    return nc


def _prep_core(x, pos_cos, pos_sin, wq, wk, wv, wo, tp, dp):
    gh = [tp * QH + h for h in PERM]
    qcols = np.concatenate([np.arange(g * HD, (g + 1) * HD) for g in gh])
    wqr = wq[:, qcols].astype(bf).reshape(DC, 128, QH * HD).copy()
    kvc = np.arange(tp * KVH * HD, (tp + 1) * KVH * HD)
    wkv = np.concatenate([wk[:, kvc], wv[:, kvc]], axis=1)   # [D, 256]
    wkvr = wkv.astype(bf).reshape(DC, 128, 2 * KVH * HD).copy()
    wor = (wo[qcols, :].astype(bf)
           .reshape(PAIRS, 128, D).transpose(1, 0, 2).copy())
    xs = x[dp * BL:(dp + 1) * BL]
    xt = (xs.reshape(BL, NT, 128, DC, 128).transpose(0, 1, 4, 3, 2)
          .reshape(NQT, 128, DC, 128).astype(bf))
    cosr = (np.tile(pos_cos, (1, QH)).astype(np.float32)
            .reshape(NT, 128, QH * HD // 2).copy())
    sinr = (np.tile(pos_sin, (1, QH)).astype(np.float32)
            .reshape(NT, 128, QH * HD // 2).copy())
    maskr = np.triu(np.ones((128, 128), np.float32)).astype(bf)
    identr = np.eye(128, dtype=np.float32).astype(bf)
    return {"xt": np.ascontiguousarray(xt), "wqr": wqr, "wkvr": wkvr,
            "wor": wor, "cosr": cosr, "sinr": sinr,
            "maskr": maskr, "identr": identr}


def make_in_maps(x, pos_cos, pos_sin, wq, wk, wv, wo):
    x = np.asarray(x, np.float32)
    pos_cos = np.asarray(pos_cos, np.float32)
    pos_sin = np.asarray(pos_sin, np.float32)
    wq = np.asarray(wq, np.float32)
    wk = np.asarray(wk, np.float32)
    wv = np.asarray(wv, np.float32)
    wo = np.asarray(wo, np.float32)
    return [_prep_core(x, pos_cos, pos_sin, wq, wk, wv, wo, c % TP, c // TP)
            for c in range(TP * DP)]


def gather(results):
    y = np.empty((B, S, D), np.float32)
    for dp in range(DP):
        acc = results[dp * TP]["y"].astype(np.float32)
        for t in range(1, TP):
            acc += results[dp * TP + t]["y"].astype(np.float32)
        y[dp * BL:(dp + 1) * BL] = acc.reshape(BL, S, D)
    return y


def get_nc():
    global _built
    if _built is None:
        _built = _build()
    return _built


def kernel(x, pos_cos, pos_sin, wq, wk, wv, wo):
    from concourse.bass_utils import run_bass_kernel_spmd
    nc = get_nc()
    in_maps = make_in_maps(x, pos_cos, pos_sin, wq, wk, wv, wo)
    res = run_bass_kernel_spmd(nc, in_maps, list(range(TP * DP)))
    return gather(res.results)
